# revision 1
# baseline (speedup 1.0000x reference)
"""GPTQ-style grouped-dequant linear on 8 Trainium2 cores.

out[m,n] = sum_k A[m,k] * (q[n,k] - zeros[n,k//128]) * scales[n,k//128] + bias[n]
M=2048, K=4096, N=4096, group=128.

Sharding: column-parallel — qweight/scales/zeros/bias split along N (512/core),
A replicated (host-transposed to [K, M] so contraction lands on partitions).
Per core: dequant q (uint8) -> bf16 W tiles in [n,k] layout via one
tensor_scalar (f32 per-partition scale/zero), PE-transpose to [k,n], then a
single PSUM-accumulated bf16 matmul chain per 128-row output tile with bias
injected as a rank-1 (ones x bias) matmul.
"""

import numpy as np

import concourse.bass as bass
import concourse.mybir as mybir
import concourse.tile as tile
from concourse import bacc
from concourse.masks import make_identity
from concourse.bass_utils import run_bass_kernel_spmd

P = 128
M, K, N = 2048, 4096, 4096
NCORES = 8
NS = N // NCORES          # 512 out-features per core
G = K // P                # 32 groups (group_size == P == 128)
MT = M // P               # 16 output row tiles
NB = NS // P              # 4 n-chunks per core

_cached = None


def _build():
    nc = bacc.Bacc("TRN2", target_bir_lowering=False, debug=False,
                   num_devices=NCORES)
    at = nc.dram_tensor("AT", [K, M], mybir.dt.float32, kind="ExternalInput")
    q8 = nc.dram_tensor("q8", [NS, K], mybir.dt.uint8, kind="ExternalInput")
    sc = nc.dram_tensor("sc", [NS, G], mybir.dt.float32, kind="ExternalInput")
    zp = nc.dram_tensor("zp", [NS, G], mybir.dt.float32, kind="ExternalInput")
    bi = nc.dram_tensor("bias", [1, NS], mybir.dt.float32, kind="ExternalInput")
    out = nc.dram_tensor("out", [M, NS], mybir.dt.float32, kind="ExternalOutput")

    bf16, f32 = mybir.dt.bfloat16, mybir.dt.float32

    with tile.TileContext(nc) as tc:
        with (
            tc.tile_pool(name="const", bufs=1) as const,
            tc.tile_pool(name="qpool", bufs=NB) as qpool,
            tc.tile_pool(name="wnk", bufs=4) as wnk,
            tc.tile_pool(name="tpsum", bufs=2, space="PSUM") as tpsum,
            tc.tile_pool(name="wt", bufs=1) as wtp,
            tc.tile_pool(name="apool", bufs=3) as apool,
            tc.tile_pool(name="abpool", bufs=3) as abpool,
            tc.tile_pool(name="mpsum", bufs=3, space="PSUM") as mpsum,
            tc.tile_pool(name="opool", bufs=3) as opool,
        ):
            ident = const.tile([P, P], bf16, tag="ident")
            make_identity(nc, ident)
            ones = const.tile([1, P], bf16, tag="ones")
            nc.vector.memset(ones, 1.0)
            bias_f = const.tile([1, NS], f32, tag="bias_f")
            nc.sync.dma_start(out=bias_f[:], in_=bi.ap()[:])
            bias_b = const.tile([1, NS], bf16, tag="bias_b")
            nc.vector.tensor_copy(bias_b[:], bias_f[:])

            scts, zpts, qstrips = [], [], []
            for nb in range(NB):
                sct = const.tile([P, G], f32, tag=f"sc{nb}")
                zpt = const.tile([P, G], f32, tag=f"zp{nb}")
                nc.sync.dma_start(out=sct[:], in_=sc.ap()[nb * P:(nb + 1) * P, :])
                nc.sync.dma_start(out=zpt[:], in_=zp.ap()[nb * P:(nb + 1) * P, :])
                scts.append(sct)
                zpts.append(zpt)
                qs = qpool.tile([P, K], mybir.dt.uint8, tag=f"q{nb}")
                # split across 2 DMA queues
                h = K // 2
                nc.sync.dma_start(out=qs[:, :h],
                                  in_=q8.ap()[nb * P:(nb + 1) * P, :h])
                nc.sync.dma_start(out=qs[:, h:],
                                  in_=q8.ap()[nb * P:(nb + 1) * P, h:])
                qstrips.append(qs)

            # dequant + transpose, g-major so each W^T[g] completes early
            wts = []
            for g in range(G):
                wt = wtp.tile([P, NS], bf16, tag=f"wt{g}")
                wts.append(wt)
                for nb in range(NB):
                    wn = wnk.tile([P, P], bf16)
                    nc.vector.tensor_scalar(
                        wn[:], qstrips[nb][:, g * P:(g + 1) * P],
                        zpts[nb][:, g:g + 1], scts[nb][:, g:g + 1],
                        mybir.AluOpType.subtract, mybir.AluOpType.mult)
                    tp = tpsum.tile([P, P], bf16)
                    nc.tensor.transpose(tp[:], wn[:], ident[:])
                    nc.vector.tensor_copy(wt[:, nb * P:(nb + 1) * P], tp[:])

            atr = at.ap().rearrange("(g p) m -> p g m", p=P)  # [128, G, M]
            for mt in range(MT):
                af = apool.tile([P, G, P], f32)
                ms = mt * P
                for h in range(4):  # 4 DMA queues x 512KB
                    g0, g1 = h * (G // 4), (h + 1) * (G // 4)
                    nc.sync.dma_start(out=af[:, g0:g1, :],
                                      in_=atr[:, g0:g1, ms:ms + P])
                ab = abpool.tile([P, G, P], bf16)
                nc.scalar.copy(ab[:], af[:])
                ps = mpsum.tile([P, NS], f32)
                nc.tensor.matmul(ps[:], ones[:], bias_b[:],
                                 start=True, stop=False)
                for g in range(G):
                    nc.tensor.matmul(ps[:], ab[:, g, :], wts[g][:],
                                     start=False, stop=(g == G - 1))
                ob = opool.tile([P, NS], f32)
                nc.scalar.copy(ob[:], ps[:])
                nc.sync.dma_start(out=out.ap()[ms:ms + P, :], in_=ob[:])

    nc.compile()
    return nc


def _prep_inputs(A, qweight, scales, zeros, bias):
    at = np.ascontiguousarray(A.T)
    in_maps = []
    for c in range(NCORES):
        r = slice(c * NS, (c + 1) * NS)
        in_maps.append({
            "AT": at,
            "q8": np.ascontiguousarray(qweight[r]).astype(np.uint8),
            "sc": np.ascontiguousarray(scales[r]),
            "zp": np.ascontiguousarray(zeros[r]),
            "bias": np.ascontiguousarray(bias[r]).reshape(1, NS),
        })
    return in_maps


def run(inputs, **spmd_kwargs):
    global _cached
    if _cached is None:
        _cached = _build()
    in_maps = _prep_inputs(**inputs)
    res = run_bass_kernel_spmd(_cached, in_maps, list(range(NCORES)),
                               **spmd_kwargs)
    outp = np.concatenate([res.results[c]["out"] for c in range(NCORES)],
                          axis=1)
    return outp, res


def kernel(**inputs):
    return run(inputs)[0]
